# revision 5
# baseline (speedup 1.0000x reference)
"""DeepSeek-style MoE layer on 8 Trainium2 NeuronCores, expert-parallel.

Strategy:
  - Routing (sigmoid gate + group-limited top-k) and dispatch indices are
    computed on host in fp32 numpy (exact reference semantics, ~0.1% of FLOPs).
  - Expert loads are split into <=512-column segments (hot experts split into
    near-equal parts) and snake-packed into S slots x 8 cores; slot capacities
    are the per-group maxima (multiple of 8), so padded compute is minimized.
  - Each core runs one hand-rolled Bass/Tile graph over its slots:
        gT = silu(w1_s^T @ buf_s)         [I, Cs]   (psum f32, bf16 in SBUF)
        hT = gT * (w3_s^T @ buf_s)        [I, Cs]
        yT = w2_s^T-blocks @ hT           [H, Cs]   (stationary w2, moving hT)
    All pools are global rings shared across slots so DMA prefetch of slot s+1
    overlaps compute of slot s; output is written transposed in bf16.
  - Combine (gather + weighted sum over the K=8 routes) happens on host.
"""

import math

import ml_dtypes
import numpy as np

import concourse.bass as bass
import concourse.mybir as mybir
import concourse.tile as tile
from concourse import bacc
from concourse.bass_utils import run_bass_kernel_spmd

# MoE config (matches the reference)
N = 2048
H = 2048
I = 1024
E = 32
K = 8
G = 8
KG = 4
C = 1024
SCALE = 2.5

M_CORES = 8
SEG_MAX = 512  # max columns per slot -> single matmul n-tile
GRAN = 8       # slot capacity granularity

BF16 = ml_dtypes.bfloat16


def _route(x, w_gate, gate_bias):
    """fp32 numpy replication of the reference gate."""
    scores = 1.0 / (1.0 + np.exp(-(x @ w_gate), dtype=np.float32))  # [N, E]
    sb = scores + gate_bias
    grp = sb.reshape(N, G, E // G)
    top2 = -np.sort(-grp, axis=-1)[..., :2]
    gscore = top2.sum(-1)  # [N, G]
    gidx = np.argsort(-gscore, axis=-1, kind="stable")[:, :KG]
    gmask = np.zeros((N, G), bool)
    gmask[np.arange(N)[:, None], gidx] = True
    emask = np.repeat(gmask, E // G, axis=1)
    masked = np.where(emask, sb, -np.inf)
    eidx = np.argsort(-masked, axis=-1, kind="stable")[:, :K]  # [N, K]
    w = np.take_along_axis(scores, eidx, axis=1)
    w = w / w.sum(-1, keepdims=True) * SCALE
    return eidx, w.astype(np.float32)


def _dispatch_indices(eidx):
    """Per-route slot positions, replicating the reference capacity rule."""
    flat_e = eidx.reshape(-1)  # [N*K], token-major arrival order
    tok = np.repeat(np.arange(N), K)
    order = np.argsort(flat_e, kind="stable")
    counts = np.bincount(flat_e, minlength=E)
    starts = np.concatenate([[0], np.cumsum(counts)[:-1]])
    pos_sorted = np.arange(N * K) - np.repeat(starts, counts)
    pos = np.empty(N * K, np.int64)
    pos[order] = pos_sorted
    valid = pos < C
    return flat_e, tok, pos, valid, counts


def _pack(counts):
    """Split expert loads into <=SEG_MAX segments, snake-pack into slots.

    Returns (caps, assign) where caps[s] is slot s's column capacity and
    assign[c][s] = (expert, start_pos, ncols) for core c, slot s.
    """
    loads = np.minimum(counts, C).astype(np.int64)
    parts = [max(1, int(math.ceil(l / SEG_MAX))) for l in loads]
    S = int(math.ceil(sum(parts) / M_CORES))
    while sum(parts) < M_CORES * S:
        e = max(range(E), key=lambda e: loads[e] / parts[e])
        parts[e] += 1
    segs = []  # (size, expert, start)
    for e in range(E):
        k = parts[e]
        base, rem = divmod(int(loads[e]), k)
        st = 0
        for j in range(k):
            sz = base + (1 if j < rem else 0)
            segs.append((sz, e, st))
            st += sz
    segs.sort(key=lambda t: -t[0])
    caps = []
    assign = [[None] * S for _ in range(M_CORES)]
    for s in range(S):
        grp = segs[M_CORES * s : M_CORES * (s + 1)]
        mx = max(g[0] for g in grp)
        caps.append(max(GRAN, int(math.ceil(mx / GRAN) * GRAN)))
        cores = range(M_CORES) if s % 2 == 0 else range(M_CORES - 1, -1, -1)
        for c, (sz, e, st) in zip(cores, grp):
            assign[c][s] = (e, st, sz)
    return caps, assign


def _build_graph(caps):
    S = len(caps)
    CT = int(sum(caps))
    offs = np.concatenate([[0], np.cumsum(caps)]).astype(np.int64)
    f32 = mybir.dt.float32
    bf = mybir.dt.bfloat16
    ds = bass.ds

    nc = bacc.Bacc(None, target_bir_lowering=False, debug=False)
    w1t = nc.declare_dram_parameter("w1t", [S, 8, 128, 16, 128], bf, isOutput=False)
    w3t = nc.declare_dram_parameter("w3t", [S, 8, 128, 16, 128], bf, isOutput=False)
    w2t = nc.declare_dram_parameter("w2t", [S, 16, 128, 8, 128], bf, isOutput=False)
    buft = nc.declare_dram_parameter("buft", [H, CT], bf, isOutput=False)
    yt = nc.declare_dram_parameter("yt", [H, CT], bf, isOutput=True)

    with tile.TileContext(nc) as tc:
        with tc.tile_pool(name="wp", bufs=20) as wp, \
             tc.tile_pool(name="w2p", bufs=10) as w2p, \
             tc.tile_pool(name="bp", bufs=24) as bp, \
             tc.tile_pool(name="ghp", bufs=2) as ghp, \
             tc.tile_pool(name="yp", bufs=12) as yp, \
             tc.tile_pool(name="pp", bufs=8, space="PSUM") as pp:
            for s in range(S):
                Cs = int(caps[s])
                o = int(offs[s])
                # buf: 16 k-block tiles [128, Cs]
                bts = []
                for k in range(16):
                    bt = bp.tile([128, SEG_MAX], bf, tag="buf", name=f"buf{s}_{k}")
                    nc.sync.dma_start(bt[:, :Cs], buft[ds(k * 128, 128), ds(o, Cs)])
                    bts.append(bt)
                gt = ghp.tile([128, 8, SEG_MAX], bf, tag="gt", name=f"gt{s}")
                ht = ghp.tile([128, 8, SEG_MAX], bf, tag="ht", name=f"ht{s}")
                for m in range(8):
                    w1m = wp.tile([128, 16, 128], bf, tag="w13", name=f"w1_{s}_{m}")
                    nc.sync.dma_start(w1m, w1t[s, m])
                    w3m = wp.tile([128, 16, 128], bf, tag="w13", name=f"w3_{s}_{m}")
                    nc.sync.dma_start(w3m, w3t[s, m])
                    p1 = pp.tile([128, SEG_MAX], f32, tag="ps", name=f"p1_{s}_{m}")
                    for k in range(16):
                        nc.tensor.matmul(
                            p1[:, :Cs], w1m[:, ds(k, 1), :], bts[k][:, :Cs],
                            start=(k == 0), stop=(k == 15),
                        )
                    nc.scalar.activation(
                        gt[:, ds(m, 1), :Cs], p1[:, :Cs],
                        mybir.ActivationFunctionType.Silu,
                    )
                    p2 = pp.tile([128, SEG_MAX], f32, tag="ps", name=f"p2_{s}_{m}")
                    for k in range(16):
                        nc.tensor.matmul(
                            p2[:, :Cs], w3m[:, ds(k, 1), :], bts[k][:, :Cs],
                            start=(k == 0), stop=(k == 15),
                        )
                    nc.vector.tensor_mul(
                        out=ht[:, ds(m, 1), :Cs], in0=p2[:, :Cs],
                        in1=gt[:, ds(m, 1), :Cs],
                    )
                for h in range(16):
                    w2h = w2p.tile([128, 8, 128], bf, tag="w2", name=f"w2_{s}_{h}")
                    nc.sync.dma_start(w2h, w2t[s, h])
                    p3 = pp.tile([128, SEG_MAX], f32, tag="ps", name=f"p3_{s}_{h}")
                    for k in range(8):
                        nc.tensor.matmul(
                            p3[:, :Cs], w2h[:, ds(k, 1), :], ht[:, ds(k, 1), :Cs],
                            start=(k == 0), stop=(k == 7),
                        )
                    yo = yp.tile([128, SEG_MAX], bf, tag="y", name=f"y_{s}_{h}")
                    nc.any.tensor_copy(out=yo[:, :Cs], in_=p3[:, :Cs])
                    nc.sync.dma_start(yt[ds(h * 128, 128), ds(o, Cs)], yo[:, :Cs])
    nc.compile()
    return nc


_GRAPH_CACHE = {}


def _prepare(x, w_gate, gate_bias, w1, w3, w2):
    """Host-side routing, packing, and per-core input staging."""
    x = np.asarray(x, np.float32)
    eidx, w = _route(x, np.asarray(w_gate, np.float32), np.asarray(gate_bias, np.float32))
    flat_e, tok, pos, valid, counts = _dispatch_indices(eidx)
    caps, assign = _pack(counts)
    S = len(caps)
    CT = int(sum(caps))
    offs = np.concatenate([[0], np.cumsum(caps)]).astype(np.int64)

    w1b = np.asarray(w1, np.float32).astype(BF16)
    w3b = np.asarray(w3, np.float32).astype(BF16)
    w2b = np.asarray(w2, np.float32).astype(BF16)
    xb = x.astype(BF16)

    # per-expert token lists in arrival order
    etoks = []
    for e in range(E):
        m = (flat_e == e) & valid
        etoks.append(tok[m])

    # route -> (core, column) lookup tables, built from segment spans
    core_of = np.zeros((E, C), np.int64)
    col_of = np.zeros((E, C), np.int64)

    in_maps = []
    for c in range(M_CORES):
        w1s = np.zeros((S, 8, 128, 16, 128), BF16)
        w3s = np.zeros((S, 8, 128, 16, 128), BF16)
        w2s = np.zeros((S, 16, 128, 8, 128), BF16)
        buf = np.zeros((H, CT), BF16)
        for s in range(S):
            e, st, sz = assign[c][s]
            # weights: w1/w3 [H, I] -> (m, p, k, i); w2 [I, H] -> (h, p, k, j)
            # w1/w3 [H, I]: (k p) h-split, (m i) I-split -> [m, p, k, i]
            w1s[s] = w1b[e].reshape(16, 128, 8, 128).transpose(2, 1, 0, 3)
            w3s[s] = w3b[e].reshape(16, 128, 8, 128).transpose(2, 1, 0, 3)
            # w2 [I, H]: (k p) I-split, (h j) H-split -> [h, p, k, j]
            w2s[s] = w2b[e].reshape(8, 128, 16, 128).transpose(2, 1, 0, 3)
            if sz > 0:
                toks = etoks[e][st : st + sz]
                o = int(offs[s])
                buf[:, o : o + sz] = xb[toks].T
                core_of[e, st : st + sz] = c
                col_of[e, st : st + sz] = o + np.arange(sz)
        in_maps.append({"w1t": w1s, "w3t": w3s, "w2t": w2s, "buft": buf})

    meta = dict(caps=caps, CT=CT, flat_e=flat_e, tok=tok, pos=pos, valid=valid,
                w=w, core_of=core_of, col_of=col_of)
    return in_maps, meta


def _combine(ys, meta):
    """ys: [M_CORES, H, CT] bf16 -> full [N, H] f32 output."""
    flat_e, pos, valid, w = meta["flat_e"], meta["pos"], meta["valid"], meta["w"]
    safe_pos = np.where(valid, pos, 0)
    core_idx = meta["core_of"][flat_e, safe_pos]
    col_idx = meta["col_of"][flat_e, safe_pos]
    contrib = ys[core_idx, :, col_idx].astype(np.float32)  # [N*K, H]
    wf = np.where(valid, w.reshape(-1), 0.0).astype(np.float32)
    out = (contrib * wf[:, None]).reshape(N, K, H).sum(axis=1)
    return out.astype(np.float32)


def kernel(x, w_gate, gate_bias, w1, w3, w2):
    in_maps, meta = _prepare(x, w_gate, gate_bias, w1, w3, w2)
    key = tuple(meta["caps"])
    if key not in _GRAPH_CACHE:
        _GRAPH_CACHE[key] = _build_graph(meta["caps"])
    nc = _GRAPH_CACHE[key]
    res = run_bass_kernel_spmd(nc, in_maps, core_ids=list(range(M_CORES)))
    ys = np.stack([np.asarray(res.results[c]["yt"]) for c in range(M_CORES)])
    return _combine(ys, meta)


# revision 7
# speedup vs baseline: 1.1136x; 1.1136x over previous
"""DeepSeek-style MoE layer on 8 Trainium2 NeuronCores, expert-parallel.

Strategy:
  - Routing (sigmoid gate + group-limited top-k) and dispatch indices are
    computed on host in fp32 numpy (exact reference semantics, ~0.1% of FLOPs).
  - Expert loads are split into <=512-column segments (hot experts split into
    near-equal parts) and snake-packed into S slots x 8 cores; slot capacities
    are the per-group maxima (multiple of 8), so padded compute is minimized.
  - Each core runs one hand-rolled Bass/Tile graph over its slots:
        gT = silu(w1_s^T @ buf_s)         [I, Cs]   (psum f32, bf16 in SBUF)
        hT = gT * (w3_s^T @ buf_s)        [I, Cs]
        yT = w2_s^T-blocks @ hT           [H, Cs]   (stationary w2, moving hT)
    All pools are global rings shared across slots so DMA prefetch of slot s+1
    overlaps compute of slot s; output is written transposed in bf16.
  - Combine (gather + weighted sum over the K=8 routes) happens on host.
"""

import math

import ml_dtypes
import numpy as np

import concourse.bass as bass
import concourse.mybir as mybir
import concourse.tile as tile
from concourse import bacc
from concourse.bass_utils import run_bass_kernel_spmd

# MoE config (matches the reference)
N = 2048
H = 2048
I = 1024
E = 32
K = 8
G = 8
KG = 4
C = 1024
SCALE = 2.5

M_CORES = 8
SEG_MAX = 560  # max columns per slot (processed as <=512-col n-tiles)
NT_MAX = 512   # max matmul n-tile width (one PSUM bank)
GRAN = 8       # slot capacity granularity


def _ntiles(Cs):
    """Split Cs columns into near-equal n-tiles of width <= NT_MAX."""
    n = int(math.ceil(Cs / NT_MAX))
    base, rem = divmod(Cs, n)
    out, n0 = [], 0
    for j in range(n):
        nt = base + (1 if j < rem else 0)
        out.append((n0, nt))
        n0 += nt
    return out

BF16 = ml_dtypes.bfloat16


def _route(x, w_gate, gate_bias):
    """fp32 numpy replication of the reference gate."""
    scores = 1.0 / (1.0 + np.exp(-(x @ w_gate), dtype=np.float32))  # [N, E]
    sb = scores + gate_bias
    grp = sb.reshape(N, G, E // G)
    top2 = -np.sort(-grp, axis=-1)[..., :2]
    gscore = top2.sum(-1)  # [N, G]
    gidx = np.argsort(-gscore, axis=-1, kind="stable")[:, :KG]
    gmask = np.zeros((N, G), bool)
    gmask[np.arange(N)[:, None], gidx] = True
    emask = np.repeat(gmask, E // G, axis=1)
    masked = np.where(emask, sb, -np.inf)
    eidx = np.argsort(-masked, axis=-1, kind="stable")[:, :K]  # [N, K]
    w = np.take_along_axis(scores, eidx, axis=1)
    w = w / w.sum(-1, keepdims=True) * SCALE
    return eidx, w.astype(np.float32)


def _dispatch_indices(eidx):
    """Per-route slot positions, replicating the reference capacity rule."""
    flat_e = eidx.reshape(-1)  # [N*K], token-major arrival order
    tok = np.repeat(np.arange(N), K)
    order = np.argsort(flat_e, kind="stable")
    counts = np.bincount(flat_e, minlength=E)
    starts = np.concatenate([[0], np.cumsum(counts)[:-1]])
    pos_sorted = np.arange(N * K) - np.repeat(starts, counts)
    pos = np.empty(N * K, np.int64)
    pos[order] = pos_sorted
    valid = pos < C
    return flat_e, tok, pos, valid, counts


def _pack(counts):
    """Split expert loads into <=SEG_MAX segments, snake-pack into slots.

    Returns (caps, assign) where caps[s] is slot s's column capacity and
    assign[c][s] = (expert, start_pos, ncols) for core c, slot s.
    """
    loads = np.minimum(counts, C).astype(np.int64)
    parts = [max(1, int(math.ceil(l / SEG_MAX))) for l in loads]
    S = int(math.ceil(sum(parts) / M_CORES))
    while sum(parts) < M_CORES * S:
        e = max(range(E), key=lambda e: loads[e] / parts[e])
        parts[e] += 1
    segs = []  # (size, expert, start)
    for e in range(E):
        k = parts[e]
        base, rem = divmod(int(loads[e]), k)
        st = 0
        for j in range(k):
            sz = base + (1 if j < rem else 0)
            segs.append((sz, e, st))
            st += sz
    segs.sort(key=lambda t: -t[0])
    caps = []
    assign = [[None] * S for _ in range(M_CORES)]
    for s in range(S):
        grp = segs[M_CORES * s : M_CORES * (s + 1)]
        mx = max(g[0] for g in grp)
        caps.append(max(GRAN, int(math.ceil(mx / GRAN) * GRAN)))
        cores = range(M_CORES) if s % 2 == 0 else range(M_CORES - 1, -1, -1)
        for c, (sz, e, st) in zip(cores, grp):
            assign[c][s] = (e, st, sz)
    return caps, assign


def _build_graph(caps):
    S = len(caps)
    CT = int(sum(caps))
    offs = np.concatenate([[0], np.cumsum(caps)]).astype(np.int64)
    f32 = mybir.dt.float32
    bf = mybir.dt.bfloat16
    ds = bass.ds

    nc = bacc.Bacc(None, target_bir_lowering=False, debug=False)
    w1t = nc.declare_dram_parameter("w1t", [S, 8, 128, 16, 128], bf, isOutput=False)
    w3t = nc.declare_dram_parameter("w3t", [S, 8, 128, 16, 128], bf, isOutput=False)
    w2t = nc.declare_dram_parameter("w2t", [S, 16, 128, 8, 128], bf, isOutput=False)
    buft = nc.declare_dram_parameter("buft", [H, CT], bf, isOutput=False)
    yt = nc.declare_dram_parameter("yt", [H, CT], bf, isOutput=True)

    with tile.TileContext(nc) as tc:
        with tc.tile_pool(name="wp", bufs=6) as wp, \
             tc.tile_pool(name="w2p", bufs=4) as w2p, \
             tc.tile_pool(name="bp", bufs=36) as bp, \
             tc.tile_pool(name="ghp", bufs=2) as ghp, \
             tc.tile_pool(name="yp", bufs=6) as yp, \
             tc.tile_pool(name="pp", bufs=8, space="PSUM") as pp:
            for s in range(S):
                Cs = int(caps[s])
                o = int(offs[s])
                nts = _ntiles(Cs)
                # buf: 16 k-block tiles [128, Cs], DMA'd per n-chunk
                bts = []
                for k in range(16):
                    bt = bp.tile([128, SEG_MAX], bf, tag="buf", name=f"buf{s}_{k}")
                    bts.append(bt)
                for n0, nt in nts:
                    for k in range(16):
                        nc.sync.dma_start(
                            bts[k][:, ds(n0, nt)],
                            buft[ds(k * 128, 128), ds(o + n0, nt)],
                        )
                gt = ghp.tile([128, 8, SEG_MAX], bf, tag="gt", name=f"gt{s}")
                ht = ghp.tile([128, 8, SEG_MAX], bf, tag="ht", name=f"ht{s}")
                for m in range(8):
                    w1m = wp.tile([128, 16, 128], bf, tag="w13", name=f"w1_{s}_{m}")
                    nc.sync.dma_start(w1m, w1t[s, m])
                    w3m = wp.tile([128, 16, 128], bf, tag="w13", name=f"w3_{s}_{m}")
                    nc.sync.dma_start(w3m, w3t[s, m])
                    for j, (n0, nt) in enumerate(nts):
                        p1 = pp.tile([128, NT_MAX], f32, tag="ps", name=f"p1_{s}_{m}_{j}")
                        for k in range(16):
                            nc.tensor.matmul(
                                p1[:, :nt], w1m[:, ds(k, 1), :],
                                bts[k][:, ds(n0, nt)],
                                start=(k == 0), stop=(k == 15),
                            )
                        nc.scalar.activation(
                            gt[:, ds(m, 1), ds(n0, nt)], p1[:, :nt],
                            mybir.ActivationFunctionType.Silu,
                        )
                        p2 = pp.tile([128, NT_MAX], f32, tag="ps", name=f"p2_{s}_{m}_{j}")
                        for k in range(16):
                            nc.tensor.matmul(
                                p2[:, :nt], w3m[:, ds(k, 1), :],
                                bts[k][:, ds(n0, nt)],
                                start=(k == 0), stop=(k == 15),
                            )
                        nc.vector.tensor_mul(
                            out=ht[:, ds(m, 1), ds(n0, nt)], in0=p2[:, :nt],
                            in1=gt[:, ds(m, 1), ds(n0, nt)],
                        )
                for h in range(16):
                    w2h = w2p.tile([128, 8, 128], bf, tag="w2", name=f"w2_{s}_{h}")
                    nc.sync.dma_start(w2h, w2t[s, h])
                    for j, (n0, nt) in enumerate(nts):
                        p3 = pp.tile([128, NT_MAX], f32, tag="ps", name=f"p3_{s}_{h}_{j}")
                        for k in range(8):
                            nc.tensor.matmul(
                                p3[:, :nt], w2h[:, ds(k, 1), :],
                                ht[:, ds(k, 1), ds(n0, nt)],
                                start=(k == 0), stop=(k == 7),
                            )
                        yo = yp.tile([128, NT_MAX], bf, tag="y", name=f"y_{s}_{h}_{j}")
                        nc.any.tensor_copy(out=yo[:, :nt], in_=p3[:, :nt])
                        nc.sync.dma_start(
                            yt[ds(h * 128, 128), ds(o + n0, nt)], yo[:, :nt],
                        )
    nc.compile()
    return nc


_GRAPH_CACHE = {}


def _prepare(x, w_gate, gate_bias, w1, w3, w2):
    """Host-side routing, packing, and per-core input staging."""
    x = np.asarray(x, np.float32)
    eidx, w = _route(x, np.asarray(w_gate, np.float32), np.asarray(gate_bias, np.float32))
    flat_e, tok, pos, valid, counts = _dispatch_indices(eidx)
    caps, assign = _pack(counts)
    S = len(caps)
    CT = int(sum(caps))
    offs = np.concatenate([[0], np.cumsum(caps)]).astype(np.int64)

    w1b = np.asarray(w1, np.float32).astype(BF16)
    w3b = np.asarray(w3, np.float32).astype(BF16)
    w2b = np.asarray(w2, np.float32).astype(BF16)
    xb = x.astype(BF16)

    # per-expert token lists in arrival order
    etoks = []
    for e in range(E):
        m = (flat_e == e) & valid
        etoks.append(tok[m])

    # route -> (core, column) lookup tables, built from segment spans
    core_of = np.zeros((E, C), np.int64)
    col_of = np.zeros((E, C), np.int64)

    in_maps = []
    for c in range(M_CORES):
        w1s = np.zeros((S, 8, 128, 16, 128), BF16)
        w3s = np.zeros((S, 8, 128, 16, 128), BF16)
        w2s = np.zeros((S, 16, 128, 8, 128), BF16)
        buf = np.zeros((H, CT), BF16)
        for s in range(S):
            e, st, sz = assign[c][s]
            # weights: w1/w3 [H, I] -> (m, p, k, i); w2 [I, H] -> (h, p, k, j)
            # w1/w3 [H, I]: (k p) h-split, (m i) I-split -> [m, p, k, i]
            w1s[s] = w1b[e].reshape(16, 128, 8, 128).transpose(2, 1, 0, 3)
            w3s[s] = w3b[e].reshape(16, 128, 8, 128).transpose(2, 1, 0, 3)
            # w2 [I, H]: (k p) I-split, (h j) H-split -> [h, p, k, j]
            w2s[s] = w2b[e].reshape(8, 128, 16, 128).transpose(2, 1, 0, 3)
            if sz > 0:
                toks = etoks[e][st : st + sz]
                o = int(offs[s])
                buf[:, o : o + sz] = xb[toks].T
                core_of[e, st : st + sz] = c
                col_of[e, st : st + sz] = o + np.arange(sz)
        in_maps.append({"w1t": w1s, "w3t": w3s, "w2t": w2s, "buft": buf})

    meta = dict(caps=caps, CT=CT, flat_e=flat_e, tok=tok, pos=pos, valid=valid,
                w=w, core_of=core_of, col_of=col_of)
    return in_maps, meta


def _combine(ys, meta):
    """ys: [M_CORES, H, CT] bf16 -> full [N, H] f32 output."""
    flat_e, pos, valid, w = meta["flat_e"], meta["pos"], meta["valid"], meta["w"]
    safe_pos = np.where(valid, pos, 0)
    core_idx = meta["core_of"][flat_e, safe_pos]
    col_idx = meta["col_of"][flat_e, safe_pos]
    contrib = ys[core_idx, :, col_idx].astype(np.float32)  # [N*K, H]
    wf = np.where(valid, w.reshape(-1), 0.0).astype(np.float32)
    out = (contrib * wf[:, None]).reshape(N, K, H).sum(axis=1)
    return out.astype(np.float32)


def kernel(x, w_gate, gate_bias, w1, w3, w2):
    in_maps, meta = _prepare(x, w_gate, gate_bias, w1, w3, w2)
    key = tuple(meta["caps"])
    if key not in _GRAPH_CACHE:
        _GRAPH_CACHE[key] = _build_graph(meta["caps"])
    nc = _GRAPH_CACHE[key]
    res = run_bass_kernel_spmd(nc, in_maps, core_ids=list(range(M_CORES)))
    ys = np.stack([np.asarray(res.results[c]["yt"]) for c in range(M_CORES)])
    return _combine(ys, meta)


# revision 10
# speedup vs baseline: 1.1632x; 1.0446x over previous
"""DeepSeek-style MoE layer on 8 Trainium2 NeuronCores, expert-parallel.

Strategy:
  - Routing (sigmoid gate + group-limited top-k) and dispatch indices are
    computed on host in fp32 numpy (exact reference semantics, ~0.1% of FLOPs).
  - Expert loads are split into <=512-column segments (hot experts split into
    near-equal parts) and snake-packed into S slots x 8 cores; slot capacities
    are the per-group maxima (multiple of 8), so padded compute is minimized.
  - Each core runs one hand-rolled Bass/Tile graph over its slots:
        gT = silu(w1_s^T @ buf_s)         [I, Cs]   (psum f32, bf16 in SBUF)
        hT = gT * (w3_s^T @ buf_s)        [I, Cs]
        yT = w2_s^T-blocks @ hT           [H, Cs]   (stationary w2, moving hT)
    All pools are global rings shared across slots so DMA prefetch of slot s+1
    overlaps compute of slot s; output is written transposed in bf16.
  - Combine (gather + weighted sum over the K=8 routes) happens on host.
"""

import math

import ml_dtypes
import numpy as np

import concourse.bass as bass
import concourse.mybir as mybir
import concourse.tile as tile
from concourse import bacc
from concourse.bass_utils import run_bass_kernel_spmd

# MoE config (matches the reference)
N = 2048
H = 2048
I = 1024
E = 32
K = 8
G = 8
KG = 4
C = 1024
SCALE = 2.5

M_CORES = 8
SEG_MAX = 560  # max columns per slot (processed as <=512-col n-tiles)
NT_MAX = 512   # max matmul n-tile width (one PSUM bank)
GRAN = 8       # slot capacity granularity


def _ntiles(Cs):
    """Split Cs columns into near-equal n-tiles of width <= NT_MAX."""
    n = int(math.ceil(Cs / NT_MAX))
    base, rem = divmod(Cs, n)
    out, n0 = [], 0
    for j in range(n):
        nt = base + (1 if j < rem else 0)
        out.append((n0, nt))
        n0 += nt
    return out

BF16 = ml_dtypes.bfloat16


def _route(x, w_gate, gate_bias):
    """fp32 numpy replication of the reference gate."""
    scores = 1.0 / (1.0 + np.exp(-(x @ w_gate), dtype=np.float32))  # [N, E]
    sb = scores + gate_bias
    grp = sb.reshape(N, G, E // G)
    top2 = -np.sort(-grp, axis=-1)[..., :2]
    gscore = top2.sum(-1)  # [N, G]
    gidx = np.argsort(-gscore, axis=-1, kind="stable")[:, :KG]
    gmask = np.zeros((N, G), bool)
    gmask[np.arange(N)[:, None], gidx] = True
    emask = np.repeat(gmask, E // G, axis=1)
    masked = np.where(emask, sb, -np.inf)
    eidx = np.argsort(-masked, axis=-1, kind="stable")[:, :K]  # [N, K]
    w = np.take_along_axis(scores, eidx, axis=1)
    w = w / w.sum(-1, keepdims=True) * SCALE
    return eidx, w.astype(np.float32)


def _dispatch_indices(eidx):
    """Per-route slot positions, replicating the reference capacity rule."""
    flat_e = eidx.reshape(-1)  # [N*K], token-major arrival order
    tok = np.repeat(np.arange(N), K)
    order = np.argsort(flat_e, kind="stable")
    counts = np.bincount(flat_e, minlength=E)
    starts = np.concatenate([[0], np.cumsum(counts)[:-1]])
    pos_sorted = np.arange(N * K) - np.repeat(starts, counts)
    pos = np.empty(N * K, np.int64)
    pos[order] = pos_sorted
    valid = pos < C
    return flat_e, tok, pos, valid, counts


def _pack(counts):
    """Split expert loads into <=SEG_MAX segments, snake-pack into slots.

    Returns (caps, assign) where caps[s] is slot s's column capacity and
    assign[c][s] = (expert, start_pos, ncols) for core c, slot s.
    """
    loads = np.minimum(counts, C).astype(np.int64)
    parts = [max(1, int(math.ceil(l / SEG_MAX))) for l in loads]
    S = int(math.ceil(sum(parts) / M_CORES))
    while sum(parts) < M_CORES * S:
        e = max(range(E), key=lambda e: loads[e] / parts[e])
        parts[e] += 1
    segs = []  # (size, expert, start)
    for e in range(E):
        k = parts[e]
        base, rem = divmod(int(loads[e]), k)
        st = 0
        for j in range(k):
            sz = base + (1 if j < rem else 0)
            segs.append((sz, e, st))
            st += sz
    segs.sort(key=lambda t: -t[0])
    caps = []
    assign = [[None] * S for _ in range(M_CORES)]
    for s in range(S):
        grp = segs[M_CORES * s : M_CORES * (s + 1)]
        mx = max(g[0] for g in grp)
        caps.append(max(GRAN, int(math.ceil(mx / GRAN) * GRAN)))
        cores = range(M_CORES) if s % 2 == 0 else range(M_CORES - 1, -1, -1)
        for c, (sz, e, st) in zip(cores, grp):
            assign[c][s] = (e, st, sz)
    # schedule: two biggest first (best compute/byte during DMA ramp), then
    # ascending so the low-slack small slots sit in the middle, not the tail
    order = list(range(S))
    if S > 3:
        order = [0, 1] + list(range(S - 1, 1, -1))
    caps = [caps[s] for s in order]
    assign = [[row[s] for s in order] for row in assign]
    return caps, assign


def _build_graph(caps):
    S = len(caps)
    CT = int(sum(caps))
    offs = np.concatenate([[0], np.cumsum(caps)]).astype(np.int64)
    f32 = mybir.dt.float32
    bf = mybir.dt.bfloat16
    ds = bass.ds

    nc = bacc.Bacc(None, target_bir_lowering=False, debug=False)
    w1t = nc.declare_dram_parameter("w1t", [S, 8, 128, 16, 128], bf, isOutput=False)
    w3t = nc.declare_dram_parameter("w3t", [S, 8, 128, 16, 128], bf, isOutput=False)
    w2t = nc.declare_dram_parameter("w2t", [S, 16, 128, 8, 128], bf, isOutput=False)
    buft = nc.declare_dram_parameter("buft", [H, CT], bf, isOutput=False)
    yt = nc.declare_dram_parameter("yt", [H, CT], bf, isOutput=True)

    with tile.TileContext(nc) as tc:
        with tc.tile_pool(name="wp", bufs=8) as wp, \
             tc.tile_pool(name="w2p", bufs=6) as w2p, \
             tc.tile_pool(name="bp", bufs=36) as bp, \
             tc.tile_pool(name="ghp", bufs=2) as ghp, \
             tc.tile_pool(name="yp", bufs=6) as yp, \
             tc.tile_pool(name="pp", bufs=8, space="PSUM") as pp:
            for s in range(S):
                Cs = int(caps[s])
                o = int(offs[s])
                nts = _ntiles(Cs)

                # DMAs are emitted in consumption order: the m=0 weight pair
                # first, then the first buf chunk, further chunks after m=1's
                # weights (queue order == emission order).
                def w13_pair(m):
                    w1m = wp.tile([128, 16, 128], bf, tag="w13", name=f"w1_{s}_{m}")
                    nc.sync.dma_start(w1m, w1t[s, m])
                    w3m = wp.tile([128, 16, 128], bf, tag="w13", name=f"w3_{s}_{m}")
                    nc.sync.dma_start(w3m, w3t[s, m])
                    return w1m, w3m

                def buf_chunk(j):
                    n0, nt = nts[j]
                    for k in range(16):
                        nc.sync.dma_start(
                            bts[k][:, ds(n0, nt)],
                            buft[ds(k * 128, 128), ds(o + n0, nt)],
                        )

                bts = []
                for k in range(16):
                    bt = bp.tile([128, SEG_MAX], bf, tag="buf", name=f"buf{s}_{k}")
                    bts.append(bt)
                wpair0 = w13_pair(0)
                for j in range(len(nts)):
                    buf_chunk(j)
                gt = ghp.tile([128, 8, SEG_MAX], bf, tag="gt", name=f"gt{s}")
                ht = ghp.tile([128, 8, SEG_MAX], bf, tag="ht", name=f"ht{s}")
                for m in range(8):
                    w1m, w3m = wpair0 if m == 0 else w13_pair(m)
                    for j, (n0, nt) in enumerate(nts):
                        p1 = pp.tile([128, NT_MAX], f32, tag="ps", name=f"p1_{s}_{m}_{j}")
                        for k in range(16):
                            nc.tensor.matmul(
                                p1[:, :nt], w1m[:, ds(k, 1), :],
                                bts[k][:, ds(n0, nt)],
                                start=(k == 0), stop=(k == 15),
                            )
                        nc.scalar.activation(
                            gt[:, ds(m, 1), ds(n0, nt)], p1[:, :nt],
                            mybir.ActivationFunctionType.Silu,
                        )
                        p2 = pp.tile([128, NT_MAX], f32, tag="ps", name=f"p2_{s}_{m}_{j}")
                        for k in range(16):
                            nc.tensor.matmul(
                                p2[:, :nt], w3m[:, ds(k, 1), :],
                                bts[k][:, ds(n0, nt)],
                                start=(k == 0), stop=(k == 15),
                            )
                        nc.vector.tensor_mul(
                            out=ht[:, ds(m, 1), ds(n0, nt)], in0=p2[:, :nt],
                            in1=gt[:, ds(m, 1), ds(n0, nt)],
                        )
                for h in range(16):
                    w2h = w2p.tile([128, 8, 128], bf, tag="w2", name=f"w2_{s}_{h}")
                    nc.sync.dma_start(w2h, w2t[s, h])
                    for j, (n0, nt) in enumerate(nts):
                        p3 = pp.tile([128, NT_MAX], f32, tag="ps", name=f"p3_{s}_{h}_{j}")
                        for k in range(8):
                            nc.tensor.matmul(
                                p3[:, :nt], w2h[:, ds(k, 1), :],
                                ht[:, ds(k, 1), ds(n0, nt)],
                                start=(k == 0), stop=(k == 7),
                            )
                        yo = yp.tile([128, NT_MAX], bf, tag="y", name=f"y_{s}_{h}_{j}")
                        nc.any.tensor_copy(out=yo[:, :nt], in_=p3[:, :nt])
                        nc.sync.dma_start(
                            yt[ds(h * 128, 128), ds(o + n0, nt)], yo[:, :nt],
                        )
    nc.compile()
    return nc


_GRAPH_CACHE = {}


def _prepare(x, w_gate, gate_bias, w1, w3, w2):
    """Host-side routing, packing, and per-core input staging."""
    x = np.asarray(x, np.float32)
    eidx, w = _route(x, np.asarray(w_gate, np.float32), np.asarray(gate_bias, np.float32))
    flat_e, tok, pos, valid, counts = _dispatch_indices(eidx)
    caps, assign = _pack(counts)
    S = len(caps)
    CT = int(sum(caps))
    offs = np.concatenate([[0], np.cumsum(caps)]).astype(np.int64)

    w1b = np.asarray(w1, np.float32).astype(BF16)
    w3b = np.asarray(w3, np.float32).astype(BF16)
    w2b = np.asarray(w2, np.float32).astype(BF16)
    xb = x.astype(BF16)

    # per-expert token lists in arrival order
    etoks = []
    for e in range(E):
        m = (flat_e == e) & valid
        etoks.append(tok[m])

    # route -> (core, column) lookup tables, built from segment spans
    core_of = np.zeros((E, C), np.int64)
    col_of = np.zeros((E, C), np.int64)

    in_maps = []
    for c in range(M_CORES):
        w1s = np.zeros((S, 8, 128, 16, 128), BF16)
        w3s = np.zeros((S, 8, 128, 16, 128), BF16)
        w2s = np.zeros((S, 16, 128, 8, 128), BF16)
        buf = np.zeros((H, CT), BF16)
        for s in range(S):
            e, st, sz = assign[c][s]
            # weights: w1/w3 [H, I] -> (m, p, k, i); w2 [I, H] -> (h, p, k, j)
            # w1/w3 [H, I]: (k p) h-split, (m i) I-split -> [m, p, k, i]
            w1s[s] = w1b[e].reshape(16, 128, 8, 128).transpose(2, 1, 0, 3)
            w3s[s] = w3b[e].reshape(16, 128, 8, 128).transpose(2, 1, 0, 3)
            # w2 [I, H]: (k p) I-split, (h j) H-split -> [h, p, k, j]
            w2s[s] = w2b[e].reshape(8, 128, 16, 128).transpose(2, 1, 0, 3)
            if sz > 0:
                toks = etoks[e][st : st + sz]
                o = int(offs[s])
                buf[:, o : o + sz] = xb[toks].T
                core_of[e, st : st + sz] = c
                col_of[e, st : st + sz] = o + np.arange(sz)
        in_maps.append({"w1t": w1s, "w3t": w3s, "w2t": w2s, "buft": buf})

    meta = dict(caps=caps, CT=CT, flat_e=flat_e, tok=tok, pos=pos, valid=valid,
                w=w, core_of=core_of, col_of=col_of)
    return in_maps, meta


def _combine(ys, meta):
    """ys: [M_CORES, H, CT] bf16 -> full [N, H] f32 output."""
    flat_e, pos, valid, w = meta["flat_e"], meta["pos"], meta["valid"], meta["w"]
    safe_pos = np.where(valid, pos, 0)
    core_idx = meta["core_of"][flat_e, safe_pos]
    col_idx = meta["col_of"][flat_e, safe_pos]
    contrib = ys[core_idx, :, col_idx].astype(np.float32)  # [N*K, H]
    wf = np.where(valid, w.reshape(-1), 0.0).astype(np.float32)
    out = (contrib * wf[:, None]).reshape(N, K, H).sum(axis=1)
    return out.astype(np.float32)


def kernel(x, w_gate, gate_bias, w1, w3, w2):
    in_maps, meta = _prepare(x, w_gate, gate_bias, w1, w3, w2)
    key = tuple(meta["caps"])
    if key not in _GRAPH_CACHE:
        _GRAPH_CACHE[key] = _build_graph(meta["caps"])
    nc = _GRAPH_CACHE[key]
    res = run_bass_kernel_spmd(nc, in_maps, core_ids=list(range(M_CORES)))
    ys = np.stack([np.asarray(res.results[c]["yt"]) for c in range(M_CORES)])
    return _combine(ys, meta)


# revision 11
# speedup vs baseline: 1.1840x; 1.0178x over previous
"""DeepSeek-style MoE layer on 8 Trainium2 NeuronCores, expert-parallel.

Strategy:
  - Routing (sigmoid gate + group-limited top-k) and dispatch indices are
    computed on host in fp32 numpy (exact reference semantics, ~0.1% of FLOPs).
  - Expert loads are split into <=512-column segments (hot experts split into
    near-equal parts) and snake-packed into S slots x 8 cores; slot capacities
    are the per-group maxima (multiple of 8), so padded compute is minimized.
  - Each core runs one hand-rolled Bass/Tile graph over its slots:
        gT = silu(w1_s^T @ buf_s)         [I, Cs]   (psum f32, bf16 in SBUF)
        hT = gT * (w3_s^T @ buf_s)        [I, Cs]
        yT = w2_s^T-blocks @ hT           [H, Cs]   (stationary w2, moving hT)
    All pools are global rings shared across slots so DMA prefetch of slot s+1
    overlaps compute of slot s; output is written transposed in bf16.
  - Combine (gather + weighted sum over the K=8 routes) happens on host.
"""

import math

import ml_dtypes
import numpy as np

import concourse.bass as bass
import concourse.mybir as mybir
import concourse.tile as tile
from concourse import bacc
from concourse.bass_utils import run_bass_kernel_spmd

# MoE config (matches the reference)
N = 2048
H = 2048
I = 1024
E = 32
K = 8
G = 8
KG = 4
C = 1024
SCALE = 2.5

M_CORES = 8
SEG_MAX = 560  # max columns per slot (processed as <=512-col n-tiles)
NT_MAX = 512   # max matmul n-tile width (one PSUM bank)
GRAN = 8       # slot capacity granularity


def _ntiles(Cs):
    """Split Cs columns into near-equal n-tiles of width <= NT_MAX."""
    n = int(math.ceil(Cs / NT_MAX))
    base, rem = divmod(Cs, n)
    out, n0 = [], 0
    for j in range(n):
        nt = base + (1 if j < rem else 0)
        out.append((n0, nt))
        n0 += nt
    return out

BF16 = ml_dtypes.bfloat16


def _route(x, w_gate, gate_bias):
    """fp32 numpy replication of the reference gate."""
    scores = 1.0 / (1.0 + np.exp(-(x @ w_gate), dtype=np.float32))  # [N, E]
    sb = scores + gate_bias
    grp = sb.reshape(N, G, E // G)
    top2 = -np.sort(-grp, axis=-1)[..., :2]
    gscore = top2.sum(-1)  # [N, G]
    gidx = np.argsort(-gscore, axis=-1, kind="stable")[:, :KG]
    gmask = np.zeros((N, G), bool)
    gmask[np.arange(N)[:, None], gidx] = True
    emask = np.repeat(gmask, E // G, axis=1)
    masked = np.where(emask, sb, -np.inf)
    eidx = np.argsort(-masked, axis=-1, kind="stable")[:, :K]  # [N, K]
    w = np.take_along_axis(scores, eidx, axis=1)
    w = w / w.sum(-1, keepdims=True) * SCALE
    return eidx, w.astype(np.float32)


def _dispatch_indices(eidx):
    """Per-route slot positions, replicating the reference capacity rule."""
    flat_e = eidx.reshape(-1)  # [N*K], token-major arrival order
    tok = np.repeat(np.arange(N), K)
    order = np.argsort(flat_e, kind="stable")
    counts = np.bincount(flat_e, minlength=E)
    starts = np.concatenate([[0], np.cumsum(counts)[:-1]])
    pos_sorted = np.arange(N * K) - np.repeat(starts, counts)
    pos = np.empty(N * K, np.int64)
    pos[order] = pos_sorted
    valid = pos < C
    return flat_e, tok, pos, valid, counts


def _pack(counts):
    """Split expert loads into <=SEG_MAX segments, snake-pack into slots.

    Returns (caps, assign) where caps[s] is slot s's column capacity and
    assign[c][s] = (expert, start_pos, ncols) for core c, slot s.
    """
    loads = np.minimum(counts, C).astype(np.int64)
    parts = [max(1, int(math.ceil(l / SEG_MAX))) for l in loads]
    S = int(math.ceil(sum(parts) / M_CORES))
    while sum(parts) < M_CORES * S:
        e = max(range(E), key=lambda e: loads[e] / parts[e])
        parts[e] += 1
    segs = []  # (size, expert, start)
    for e in range(E):
        k = parts[e]
        base, rem = divmod(int(loads[e]), k)
        st = 0
        for j in range(k):
            sz = base + (1 if j < rem else 0)
            segs.append((sz, e, st))
            st += sz
    segs.sort(key=lambda t: -t[0])
    caps = []
    assign = [[None] * S for _ in range(M_CORES)]
    for s in range(S):
        grp = segs[M_CORES * s : M_CORES * (s + 1)]
        mx = max(g[0] for g in grp)
        caps.append(max(GRAN, int(math.ceil(mx / GRAN) * GRAN)))
        cores = range(M_CORES) if s % 2 == 0 else range(M_CORES - 1, -1, -1)
        for c, (sz, e, st) in zip(cores, grp):
            assign[c][s] = (e, st, sz)
    # schedule: two biggest first (best compute/byte during DMA ramp), then
    # ascending so the low-slack small slots sit in the middle, not the tail
    order = list(range(S))
    if S > 3:
        order = [0, 1] + list(range(S - 1, 1, -1))
    caps = [caps[s] for s in order]
    assign = [[row[s] for s in order] for row in assign]
    return caps, assign


def _build_graph(caps):
    S = len(caps)
    CT = int(sum(caps))
    offs = np.concatenate([[0], np.cumsum(caps)]).astype(np.int64)
    f32 = mybir.dt.float32
    bf = mybir.dt.bfloat16
    ds = bass.ds

    nc = bacc.Bacc(None, target_bir_lowering=False, debug=False)
    w1t = nc.declare_dram_parameter("w1t", [S, 8, 128, 16, 128], bf, isOutput=False)
    w3t = nc.declare_dram_parameter("w3t", [S, 8, 128, 16, 128], bf, isOutput=False)
    w2t = nc.declare_dram_parameter("w2t", [S, 16, 128, 8, 128], bf, isOutput=False)
    buft = nc.declare_dram_parameter("buft", [H, CT], bf, isOutput=False)
    yt = nc.declare_dram_parameter("yt", [H, CT], bf, isOutput=True)

    with tile.TileContext(nc) as tc:
        with tc.tile_pool(name="wp", bufs=14) as wp, \
             tc.tile_pool(name="w2p", bufs=9) as w2p, \
             tc.tile_pool(name="bp", bufs=36) as bp, \
             tc.tile_pool(name="ghp", bufs=2) as ghp, \
             tc.tile_pool(name="yp", bufs=8) as yp, \
             tc.tile_pool(name="pp", bufs=8, space="PSUM") as pp:
            for s in range(S):
                Cs = int(caps[s])
                o = int(offs[s])
                nts = _ntiles(Cs)

                # DMAs are emitted in consumption order: the m=0 weight pair
                # first, then the first buf chunk, further chunks after m=1's
                # weights (queue order == emission order).
                def w13_pair(m):
                    w1m = wp.tile([128, 16, 128], bf, tag="w13", name=f"w1_{s}_{m}")
                    nc.sync.dma_start(w1m, w1t[s, m])
                    w3m = wp.tile([128, 16, 128], bf, tag="w13", name=f"w3_{s}_{m}")
                    nc.sync.dma_start(w3m, w3t[s, m])
                    return w1m, w3m

                def buf_chunk(j):
                    n0, nt = nts[j]
                    for k in range(16):
                        nc.sync.dma_start(
                            bts[k][:, ds(n0, nt)],
                            buft[ds(k * 128, 128), ds(o + n0, nt)],
                        )

                bts = []
                for k in range(16):
                    bt = bp.tile([128, SEG_MAX], bf, tag="buf", name=f"buf{s}_{k}")
                    bts.append(bt)
                wpair0 = w13_pair(0)
                for j in range(len(nts)):
                    buf_chunk(j)
                gt = ghp.tile([128, 8, SEG_MAX], bf, tag="gt", name=f"gt{s}")
                ht = ghp.tile([128, 8, SEG_MAX], bf, tag="ht", name=f"ht{s}")
                for m in range(8):
                    w1m, w3m = wpair0 if m == 0 else w13_pair(m)
                    for j, (n0, nt) in enumerate(nts):
                        p1 = pp.tile([128, NT_MAX], f32, tag="ps", name=f"p1_{s}_{m}_{j}")
                        for k in range(16):
                            nc.tensor.matmul(
                                p1[:, :nt], w1m[:, ds(k, 1), :],
                                bts[k][:, ds(n0, nt)],
                                start=(k == 0), stop=(k == 15),
                            )
                        nc.scalar.activation(
                            gt[:, ds(m, 1), ds(n0, nt)], p1[:, :nt],
                            mybir.ActivationFunctionType.Silu,
                        )
                        p2 = pp.tile([128, NT_MAX], f32, tag="ps", name=f"p2_{s}_{m}_{j}")
                        for k in range(16):
                            nc.tensor.matmul(
                                p2[:, :nt], w3m[:, ds(k, 1), :],
                                bts[k][:, ds(n0, nt)],
                                start=(k == 0), stop=(k == 15),
                            )
                        nc.vector.tensor_mul(
                            out=ht[:, ds(m, 1), ds(n0, nt)], in0=p2[:, :nt],
                            in1=gt[:, ds(m, 1), ds(n0, nt)],
                        )
                for h in range(16):
                    w2h = w2p.tile([128, 8, 128], bf, tag="w2", name=f"w2_{s}_{h}")
                    nc.sync.dma_start(w2h, w2t[s, h])
                    for j, (n0, nt) in enumerate(nts):
                        p3 = pp.tile([128, NT_MAX], f32, tag="ps", name=f"p3_{s}_{h}_{j}")
                        for k in range(8):
                            nc.tensor.matmul(
                                p3[:, :nt], w2h[:, ds(k, 1), :],
                                ht[:, ds(k, 1), ds(n0, nt)],
                                start=(k == 0), stop=(k == 7),
                            )
                        yo = yp.tile([128, NT_MAX], bf, tag="y", name=f"y_{s}_{h}_{j}")
                        nc.any.tensor_copy(out=yo[:, :nt], in_=p3[:, :nt])
                        nc.sync.dma_start(
                            yt[ds(h * 128, 128), ds(o + n0, nt)], yo[:, :nt],
                        )
    nc.compile()
    return nc


_GRAPH_CACHE = {}


def _prepare(x, w_gate, gate_bias, w1, w3, w2):
    """Host-side routing, packing, and per-core input staging."""
    x = np.asarray(x, np.float32)
    eidx, w = _route(x, np.asarray(w_gate, np.float32), np.asarray(gate_bias, np.float32))
    flat_e, tok, pos, valid, counts = _dispatch_indices(eidx)
    caps, assign = _pack(counts)
    S = len(caps)
    CT = int(sum(caps))
    offs = np.concatenate([[0], np.cumsum(caps)]).astype(np.int64)

    w1b = np.asarray(w1, np.float32).astype(BF16)
    w3b = np.asarray(w3, np.float32).astype(BF16)
    w2b = np.asarray(w2, np.float32).astype(BF16)
    xb = x.astype(BF16)

    # per-expert token lists in arrival order
    etoks = []
    for e in range(E):
        m = (flat_e == e) & valid
        etoks.append(tok[m])

    # route -> (core, column) lookup tables, built from segment spans
    core_of = np.zeros((E, C), np.int64)
    col_of = np.zeros((E, C), np.int64)

    in_maps = []
    for c in range(M_CORES):
        w1s = np.zeros((S, 8, 128, 16, 128), BF16)
        w3s = np.zeros((S, 8, 128, 16, 128), BF16)
        w2s = np.zeros((S, 16, 128, 8, 128), BF16)
        buf = np.zeros((H, CT), BF16)
        for s in range(S):
            e, st, sz = assign[c][s]
            # weights: w1/w3 [H, I] -> (m, p, k, i); w2 [I, H] -> (h, p, k, j)
            # w1/w3 [H, I]: (k p) h-split, (m i) I-split -> [m, p, k, i]
            w1s[s] = w1b[e].reshape(16, 128, 8, 128).transpose(2, 1, 0, 3)
            w3s[s] = w3b[e].reshape(16, 128, 8, 128).transpose(2, 1, 0, 3)
            # w2 [I, H]: (k p) I-split, (h j) H-split -> [h, p, k, j]
            w2s[s] = w2b[e].reshape(8, 128, 16, 128).transpose(2, 1, 0, 3)
            if sz > 0:
                toks = etoks[e][st : st + sz]
                o = int(offs[s])
                buf[:, o : o + sz] = xb[toks].T
                core_of[e, st : st + sz] = c
                col_of[e, st : st + sz] = o + np.arange(sz)
        in_maps.append({"w1t": w1s, "w3t": w3s, "w2t": w2s, "buft": buf})

    meta = dict(caps=caps, CT=CT, flat_e=flat_e, tok=tok, pos=pos, valid=valid,
                w=w, core_of=core_of, col_of=col_of)
    return in_maps, meta


def _combine(ys, meta):
    """ys: [M_CORES, H, CT] bf16 -> full [N, H] f32 output."""
    flat_e, pos, valid, w = meta["flat_e"], meta["pos"], meta["valid"], meta["w"]
    safe_pos = np.where(valid, pos, 0)
    core_idx = meta["core_of"][flat_e, safe_pos]
    col_idx = meta["col_of"][flat_e, safe_pos]
    contrib = ys[core_idx, :, col_idx].astype(np.float32)  # [N*K, H]
    wf = np.where(valid, w.reshape(-1), 0.0).astype(np.float32)
    out = (contrib * wf[:, None]).reshape(N, K, H).sum(axis=1)
    return out.astype(np.float32)


def kernel(x, w_gate, gate_bias, w1, w3, w2):
    in_maps, meta = _prepare(x, w_gate, gate_bias, w1, w3, w2)
    key = tuple(meta["caps"])
    if key not in _GRAPH_CACHE:
        _GRAPH_CACHE[key] = _build_graph(meta["caps"])
    nc = _GRAPH_CACHE[key]
    res = run_bass_kernel_spmd(nc, in_maps, core_ids=list(range(M_CORES)))
    ys = np.stack([np.asarray(res.results[c]["yt"]) for c in range(M_CORES)])
    return _combine(ys, meta)


# revision 12
# speedup vs baseline: 1.1891x; 1.0043x over previous
"""DeepSeek-style MoE layer on 8 Trainium2 NeuronCores, expert-parallel.

Strategy:
  - Routing (sigmoid gate + group-limited top-k) and dispatch indices are
    computed on host in fp32 numpy (exact reference semantics, ~0.1% of FLOPs).
  - Expert loads are split into <=512-column segments (hot experts split into
    near-equal parts) and snake-packed into S slots x 8 cores; slot capacities
    are the per-group maxima (multiple of 8), so padded compute is minimized.
  - Each core runs one hand-rolled Bass/Tile graph over its slots:
        gT = silu(w1_s^T @ buf_s)         [I, Cs]   (psum f32, bf16 in SBUF)
        hT = gT * (w3_s^T @ buf_s)        [I, Cs]
        yT = w2_s^T-blocks @ hT           [H, Cs]   (stationary w2, moving hT)
    All pools are global rings shared across slots so DMA prefetch of slot s+1
    overlaps compute of slot s; output is written transposed in bf16.
  - Combine (gather + weighted sum over the K=8 routes) happens on host.
"""

import math

import ml_dtypes
import numpy as np

import concourse.bass as bass
import concourse.mybir as mybir
import concourse.tile as tile
from concourse import bacc
from concourse.bass_utils import run_bass_kernel_spmd

# MoE config (matches the reference)
N = 2048
H = 2048
I = 1024
E = 32
K = 8
G = 8
KG = 4
C = 1024
SCALE = 2.5

M_CORES = 8
SEG_MAX = 560  # max columns per slot (processed as <=512-col n-tiles)
NT_MAX = 512   # max matmul n-tile width (one PSUM bank)
GRAN = 8       # slot capacity granularity


def _ntiles(Cs):
    """Split Cs columns into near-equal n-tiles of width <= NT_MAX."""
    n = int(math.ceil(Cs / NT_MAX))
    base, rem = divmod(Cs, n)
    out, n0 = [], 0
    for j in range(n):
        nt = base + (1 if j < rem else 0)
        out.append((n0, nt))
        n0 += nt
    return out

BF16 = ml_dtypes.bfloat16


def _route(x, w_gate, gate_bias):
    """fp32 numpy replication of the reference gate."""
    scores = 1.0 / (1.0 + np.exp(-(x @ w_gate), dtype=np.float32))  # [N, E]
    sb = scores + gate_bias
    grp = sb.reshape(N, G, E // G)
    top2 = -np.sort(-grp, axis=-1)[..., :2]
    gscore = top2.sum(-1)  # [N, G]
    gidx = np.argsort(-gscore, axis=-1, kind="stable")[:, :KG]
    gmask = np.zeros((N, G), bool)
    gmask[np.arange(N)[:, None], gidx] = True
    emask = np.repeat(gmask, E // G, axis=1)
    masked = np.where(emask, sb, -np.inf)
    eidx = np.argsort(-masked, axis=-1, kind="stable")[:, :K]  # [N, K]
    w = np.take_along_axis(scores, eidx, axis=1)
    w = w / w.sum(-1, keepdims=True) * SCALE
    return eidx, w.astype(np.float32)


def _dispatch_indices(eidx):
    """Per-route slot positions, replicating the reference capacity rule."""
    flat_e = eidx.reshape(-1)  # [N*K], token-major arrival order
    tok = np.repeat(np.arange(N), K)
    order = np.argsort(flat_e, kind="stable")
    counts = np.bincount(flat_e, minlength=E)
    starts = np.concatenate([[0], np.cumsum(counts)[:-1]])
    pos_sorted = np.arange(N * K) - np.repeat(starts, counts)
    pos = np.empty(N * K, np.int64)
    pos[order] = pos_sorted
    valid = pos < C
    return flat_e, tok, pos, valid, counts


def _pack(counts):
    """Split expert loads into <=SEG_MAX segments, snake-pack into slots.

    Returns (caps, assign) where caps[s] is slot s's column capacity and
    assign[c][s] = (expert, start_pos, ncols) for core c, slot s.
    """
    loads = np.minimum(counts, C).astype(np.int64)
    parts = [max(1, int(math.ceil(l / SEG_MAX))) for l in loads]
    S = int(math.ceil(sum(parts) / M_CORES))
    while sum(parts) < M_CORES * S:
        e = max(range(E), key=lambda e: loads[e] / parts[e])
        parts[e] += 1
    segs = []  # (size, expert, start)
    for e in range(E):
        k = parts[e]
        base, rem = divmod(int(loads[e]), k)
        st = 0
        for j in range(k):
            sz = base + (1 if j < rem else 0)
            segs.append((sz, e, st))
            st += sz
    segs.sort(key=lambda t: -t[0])
    caps = []
    assign = [[None] * S for _ in range(M_CORES)]
    for s in range(S):
        grp = segs[M_CORES * s : M_CORES * (s + 1)]
        mx = max(g[0] for g in grp)
        caps.append(max(GRAN, int(math.ceil(mx / GRAN) * GRAN)))
        cores = range(M_CORES) if s % 2 == 0 else range(M_CORES - 1, -1, -1)
        for c, (sz, e, st) in zip(cores, grp):
            assign[c][s] = (e, st, sz)
    # schedule: two biggest first (best compute/byte during DMA ramp), then
    # ascending so the low-slack small slots sit in the middle, not the tail
    order = list(range(S))
    if S > 3:
        order = [0, 1] + list(range(S - 1, 1, -1))
    caps = [caps[s] for s in order]
    assign = [[row[s] for s in order] for row in assign]
    return caps, assign


def _build_graph(caps):
    S = len(caps)
    CT = int(sum(caps))
    offs = np.concatenate([[0], np.cumsum(caps)]).astype(np.int64)
    f32 = mybir.dt.float32
    bf = mybir.dt.bfloat16
    ds = bass.ds

    nc = bacc.Bacc(None, target_bir_lowering=False, debug=False)
    w1t = nc.declare_dram_parameter("w1t", [S, 8, 128, 16, 128], bf, isOutput=False)
    w3t = nc.declare_dram_parameter("w3t", [S, 8, 128, 16, 128], bf, isOutput=False)
    w2t = nc.declare_dram_parameter("w2t", [S, 16, 128, 8, 128], bf, isOutput=False)
    buft = nc.declare_dram_parameter("buft", [H, CT], bf, isOutput=False)
    yt = nc.declare_dram_parameter("yt", [H, CT], bf, isOutput=True)

    with tile.TileContext(nc) as tc:
        with tc.tile_pool(name="wp", bufs=18) as wp, \
             tc.tile_pool(name="w2p", bufs=9) as w2p, \
             tc.tile_pool(name="bp", bufs=44) as bp, \
             tc.tile_pool(name="ghp", bufs=2) as ghp, \
             tc.tile_pool(name="yp", bufs=12) as yp, \
             tc.tile_pool(name="pp", bufs=8, space="PSUM") as pp:
            for s in range(S):
                Cs = int(caps[s])
                o = int(offs[s])
                nts = _ntiles(Cs)

                # DMAs are emitted in consumption order: the m=0 weight pair
                # first, then the first buf chunk, further chunks after m=1's
                # weights (queue order == emission order).
                def w13_pair(m):
                    w1m = wp.tile([128, 16, 128], bf, tag="w13", name=f"w1_{s}_{m}")
                    nc.sync.dma_start(w1m, w1t[s, m])
                    w3m = wp.tile([128, 16, 128], bf, tag="w13", name=f"w3_{s}_{m}")
                    nc.sync.dma_start(w3m, w3t[s, m])
                    return w1m, w3m

                def buf_chunk(j):
                    n0, nt = nts[j]
                    for k in range(16):
                        nc.sync.dma_start(
                            bts[k][:, ds(n0, nt)],
                            buft[ds(k * 128, 128), ds(o + n0, nt)],
                        )

                bts = []
                for k in range(16):
                    bt = bp.tile([128, SEG_MAX], bf, tag="buf", name=f"buf{s}_{k}")
                    bts.append(bt)
                wpair0 = w13_pair(0)
                for j in range(len(nts)):
                    buf_chunk(j)
                gt = ghp.tile([128, 8, SEG_MAX], bf, tag="gt", name=f"gt{s}")
                ht = ghp.tile([128, 8, SEG_MAX], bf, tag="ht", name=f"ht{s}")
                for m in range(8):
                    w1m, w3m = wpair0 if m == 0 else w13_pair(m)
                    for j, (n0, nt) in enumerate(nts):
                        p1 = pp.tile([128, NT_MAX], f32, tag="ps", name=f"p1_{s}_{m}_{j}")
                        for k in range(16):
                            nc.tensor.matmul(
                                p1[:, :nt], w1m[:, ds(k, 1), :],
                                bts[k][:, ds(n0, nt)],
                                start=(k == 0), stop=(k == 15),
                            )
                        nc.scalar.activation(
                            gt[:, ds(m, 1), ds(n0, nt)], p1[:, :nt],
                            mybir.ActivationFunctionType.Silu,
                        )
                        p2 = pp.tile([128, NT_MAX], f32, tag="ps", name=f"p2_{s}_{m}_{j}")
                        for k in range(16):
                            nc.tensor.matmul(
                                p2[:, :nt], w3m[:, ds(k, 1), :],
                                bts[k][:, ds(n0, nt)],
                                start=(k == 0), stop=(k == 15),
                            )
                        nc.vector.tensor_mul(
                            out=ht[:, ds(m, 1), ds(n0, nt)], in0=p2[:, :nt],
                            in1=gt[:, ds(m, 1), ds(n0, nt)],
                        )
                for h in range(16):
                    w2h = w2p.tile([128, 8, 128], bf, tag="w2", name=f"w2_{s}_{h}")
                    nc.sync.dma_start(w2h, w2t[s, h])
                    for j, (n0, nt) in enumerate(nts):
                        p3 = pp.tile([128, NT_MAX], f32, tag="ps", name=f"p3_{s}_{h}_{j}")
                        for k in range(8):
                            nc.tensor.matmul(
                                p3[:, :nt], w2h[:, ds(k, 1), :],
                                ht[:, ds(k, 1), ds(n0, nt)],
                                start=(k == 0), stop=(k == 7),
                            )
                        yo = yp.tile([128, NT_MAX], bf, tag="y", name=f"y_{s}_{h}_{j}")
                        nc.any.tensor_copy(out=yo[:, :nt], in_=p3[:, :nt])
                        nc.sync.dma_start(
                            yt[ds(h * 128, 128), ds(o + n0, nt)], yo[:, :nt],
                        )
    nc.compile()
    return nc


_GRAPH_CACHE = {}


def _prepare(x, w_gate, gate_bias, w1, w3, w2):
    """Host-side routing, packing, and per-core input staging."""
    x = np.asarray(x, np.float32)
    eidx, w = _route(x, np.asarray(w_gate, np.float32), np.asarray(gate_bias, np.float32))
    flat_e, tok, pos, valid, counts = _dispatch_indices(eidx)
    caps, assign = _pack(counts)
    S = len(caps)
    CT = int(sum(caps))
    offs = np.concatenate([[0], np.cumsum(caps)]).astype(np.int64)

    w1b = np.asarray(w1, np.float32).astype(BF16)
    w3b = np.asarray(w3, np.float32).astype(BF16)
    w2b = np.asarray(w2, np.float32).astype(BF16)
    xb = x.astype(BF16)

    # per-expert token lists in arrival order
    etoks = []
    for e in range(E):
        m = (flat_e == e) & valid
        etoks.append(tok[m])

    # route -> (core, column) lookup tables, built from segment spans
    core_of = np.zeros((E, C), np.int64)
    col_of = np.zeros((E, C), np.int64)

    in_maps = []
    for c in range(M_CORES):
        w1s = np.zeros((S, 8, 128, 16, 128), BF16)
        w3s = np.zeros((S, 8, 128, 16, 128), BF16)
        w2s = np.zeros((S, 16, 128, 8, 128), BF16)
        buf = np.zeros((H, CT), BF16)
        for s in range(S):
            e, st, sz = assign[c][s]
            # weights: w1/w3 [H, I] -> (m, p, k, i); w2 [I, H] -> (h, p, k, j)
            # w1/w3 [H, I]: (k p) h-split, (m i) I-split -> [m, p, k, i]
            w1s[s] = w1b[e].reshape(16, 128, 8, 128).transpose(2, 1, 0, 3)
            w3s[s] = w3b[e].reshape(16, 128, 8, 128).transpose(2, 1, 0, 3)
            # w2 [I, H]: (k p) I-split, (h j) H-split -> [h, p, k, j]
            w2s[s] = w2b[e].reshape(8, 128, 16, 128).transpose(2, 1, 0, 3)
            if sz > 0:
                toks = etoks[e][st : st + sz]
                o = int(offs[s])
                buf[:, o : o + sz] = xb[toks].T
                core_of[e, st : st + sz] = c
                col_of[e, st : st + sz] = o + np.arange(sz)
        in_maps.append({"w1t": w1s, "w3t": w3s, "w2t": w2s, "buft": buf})

    meta = dict(caps=caps, CT=CT, flat_e=flat_e, tok=tok, pos=pos, valid=valid,
                w=w, core_of=core_of, col_of=col_of)
    return in_maps, meta


def _combine(ys, meta):
    """ys: [M_CORES, H, CT] bf16 -> full [N, H] f32 output."""
    flat_e, pos, valid, w = meta["flat_e"], meta["pos"], meta["valid"], meta["w"]
    safe_pos = np.where(valid, pos, 0)
    core_idx = meta["core_of"][flat_e, safe_pos]
    col_idx = meta["col_of"][flat_e, safe_pos]
    contrib = ys[core_idx, :, col_idx].astype(np.float32)  # [N*K, H]
    wf = np.where(valid, w.reshape(-1), 0.0).astype(np.float32)
    out = (contrib * wf[:, None]).reshape(N, K, H).sum(axis=1)
    return out.astype(np.float32)


def kernel(x, w_gate, gate_bias, w1, w3, w2):
    in_maps, meta = _prepare(x, w_gate, gate_bias, w1, w3, w2)
    key = tuple(meta["caps"])
    if key not in _GRAPH_CACHE:
        _GRAPH_CACHE[key] = _build_graph(meta["caps"])
    nc = _GRAPH_CACHE[key]
    res = run_bass_kernel_spmd(nc, in_maps, core_ids=list(range(M_CORES)))
    ys = np.stack([np.asarray(res.results[c]["yt"]) for c in range(M_CORES)])
    return _combine(ys, meta)


# revision 13
# speedup vs baseline: 1.1978x; 1.0073x over previous
"""DeepSeek-style MoE layer on 8 Trainium2 NeuronCores, expert-parallel.

Strategy:
  - Routing (sigmoid gate + group-limited top-k) and dispatch indices are
    computed on host in fp32 numpy (exact reference semantics, ~0.1% of FLOPs).
  - Expert loads are split into <=512-column segments (hot experts split into
    near-equal parts) and snake-packed into S slots x 8 cores; slot capacities
    are the per-group maxima (multiple of 8), so padded compute is minimized.
  - Each core runs one hand-rolled Bass/Tile graph over its slots:
        gT = silu(w1_s^T @ buf_s)         [I, Cs]   (psum f32, bf16 in SBUF)
        hT = gT * (w3_s^T @ buf_s)        [I, Cs]
        yT = w2_s^T-blocks @ hT           [H, Cs]   (stationary w2, moving hT)
    All pools are global rings shared across slots so DMA prefetch of slot s+1
    overlaps compute of slot s; output is written transposed in bf16.
  - Combine (gather + weighted sum over the K=8 routes) happens on host.
"""

import math

import ml_dtypes
import numpy as np

import concourse.bass as bass
import concourse.mybir as mybir
import concourse.tile as tile
from concourse import bacc
from concourse.bass_utils import run_bass_kernel_spmd

# MoE config (matches the reference)
N = 2048
H = 2048
I = 1024
E = 32
K = 8
G = 8
KG = 4
C = 1024
SCALE = 2.5

M_CORES = 8
SEG_MAX = 560  # max columns per slot (processed as <=512-col n-tiles)
NT_MAX = 512   # max matmul n-tile width (one PSUM bank)
GRAN = 2       # slot capacity granularity (keeps 4B-aligned bf16 lines)


def _ntiles(Cs):
    """Split Cs columns into near-equal n-tiles of width <= NT_MAX."""
    n = int(math.ceil(Cs / NT_MAX))
    base, rem = divmod(Cs, n)
    out, n0 = [], 0
    for j in range(n):
        nt = base + (1 if j < rem else 0)
        out.append((n0, nt))
        n0 += nt
    return out

BF16 = ml_dtypes.bfloat16


def _route(x, w_gate, gate_bias):
    """fp32 numpy replication of the reference gate."""
    scores = 1.0 / (1.0 + np.exp(-(x @ w_gate), dtype=np.float32))  # [N, E]
    sb = scores + gate_bias
    grp = sb.reshape(N, G, E // G)
    top2 = -np.sort(-grp, axis=-1)[..., :2]
    gscore = top2.sum(-1)  # [N, G]
    gidx = np.argsort(-gscore, axis=-1, kind="stable")[:, :KG]
    gmask = np.zeros((N, G), bool)
    gmask[np.arange(N)[:, None], gidx] = True
    emask = np.repeat(gmask, E // G, axis=1)
    masked = np.where(emask, sb, -np.inf)
    eidx = np.argsort(-masked, axis=-1, kind="stable")[:, :K]  # [N, K]
    w = np.take_along_axis(scores, eidx, axis=1)
    w = w / w.sum(-1, keepdims=True) * SCALE
    return eidx, w.astype(np.float32)


def _dispatch_indices(eidx):
    """Per-route slot positions, replicating the reference capacity rule."""
    flat_e = eidx.reshape(-1)  # [N*K], token-major arrival order
    tok = np.repeat(np.arange(N), K)
    order = np.argsort(flat_e, kind="stable")
    counts = np.bincount(flat_e, minlength=E)
    starts = np.concatenate([[0], np.cumsum(counts)[:-1]])
    pos_sorted = np.arange(N * K) - np.repeat(starts, counts)
    pos = np.empty(N * K, np.int64)
    pos[order] = pos_sorted
    valid = pos < C
    return flat_e, tok, pos, valid, counts


def _pack(counts):
    """Split expert loads into <=SEG_MAX segments, snake-pack into slots.

    Returns (caps, assign) where caps[s] is slot s's column capacity and
    assign[c][s] = (expert, start_pos, ncols) for core c, slot s.
    """
    loads = np.minimum(counts, C).astype(np.int64)
    parts = [max(1, int(math.ceil(l / SEG_MAX))) for l in loads]
    S = int(math.ceil(sum(parts) / M_CORES))
    while sum(parts) < M_CORES * S:
        e = max(range(E), key=lambda e: loads[e] / parts[e])
        parts[e] += 1
    segs = []  # (size, expert, start)
    for e in range(E):
        k = parts[e]
        base, rem = divmod(int(loads[e]), k)
        st = 0
        for j in range(k):
            sz = base + (1 if j < rem else 0)
            segs.append((sz, e, st))
            st += sz
    segs.sort(key=lambda t: -t[0])
    caps = []
    assign = [[None] * S for _ in range(M_CORES)]
    for s in range(S):
        grp = segs[M_CORES * s : M_CORES * (s + 1)]
        mx = max(g[0] for g in grp)
        caps.append(max(GRAN, int(math.ceil(mx / GRAN) * GRAN)))
        cores = range(M_CORES) if s % 2 == 0 else range(M_CORES - 1, -1, -1)
        for c, (sz, e, st) in zip(cores, grp):
            assign[c][s] = (e, st, sz)
    # schedule: two biggest first (best compute/byte during DMA ramp), then
    # ascending so the low-slack small slots sit in the middle, not the tail
    order = list(range(S))
    if S > 3:
        order = [0, 1] + list(range(S - 1, 1, -1))
    caps = [caps[s] for s in order]
    assign = [[row[s] for s in order] for row in assign]
    return caps, assign


def _build_graph(caps):
    S = len(caps)
    CT = int(sum(caps))
    offs = np.concatenate([[0], np.cumsum(caps)]).astype(np.int64)
    f32 = mybir.dt.float32
    bf = mybir.dt.bfloat16
    ds = bass.ds

    nc = bacc.Bacc(None, target_bir_lowering=False, debug=False)
    w1t = nc.declare_dram_parameter("w1t", [S, 8, 128, 16, 128], bf, isOutput=False)
    w3t = nc.declare_dram_parameter("w3t", [S, 8, 128, 16, 128], bf, isOutput=False)
    w2t = nc.declare_dram_parameter("w2t", [S, 16, 128, 8, 128], bf, isOutput=False)
    buft = nc.declare_dram_parameter("buft", [H, CT], bf, isOutput=False)
    yt = nc.declare_dram_parameter("yt", [H, CT], bf, isOutput=True)

    with tile.TileContext(nc) as tc:
        with tc.tile_pool(name="wp", bufs=18) as wp, \
             tc.tile_pool(name="w2p", bufs=9) as w2p, \
             tc.tile_pool(name="bp", bufs=44) as bp, \
             tc.tile_pool(name="ghp", bufs=2) as ghp, \
             tc.tile_pool(name="yp", bufs=12) as yp, \
             tc.tile_pool(name="pp", bufs=8, space="PSUM") as pp:
            for s in range(S):
                Cs = int(caps[s])
                o = int(offs[s])
                nts = _ntiles(Cs)

                # DMAs are emitted in consumption order: the m=0 weight pair
                # first, then the first buf chunk, further chunks after m=1's
                # weights (queue order == emission order).
                def w13_pair(m):
                    w1m = wp.tile([128, 16, 128], bf, tag="w13", name=f"w1_{s}_{m}")
                    nc.sync.dma_start(w1m, w1t[s, m])
                    w3m = wp.tile([128, 16, 128], bf, tag="w13", name=f"w3_{s}_{m}")
                    nc.sync.dma_start(w3m, w3t[s, m])
                    return w1m, w3m

                def buf_chunk(j):
                    n0, nt = nts[j]
                    for k in range(16):
                        nc.sync.dma_start(
                            bts[k][:, ds(n0, nt)],
                            buft[ds(k * 128, 128), ds(o + n0, nt)],
                        )

                bts = []
                for k in range(16):
                    bt = bp.tile([128, SEG_MAX], bf, tag="buf", name=f"buf{s}_{k}")
                    bts.append(bt)
                wpair0 = w13_pair(0)
                for j in range(len(nts)):
                    buf_chunk(j)
                gt = ghp.tile([128, 8, SEG_MAX], bf, tag="gt", name=f"gt{s}")
                ht = ghp.tile([128, 8, SEG_MAX], bf, tag="ht", name=f"ht{s}")
                for m in range(8):
                    w1m, w3m = wpair0 if m == 0 else w13_pair(m)
                    for j, (n0, nt) in enumerate(nts):
                        p1 = pp.tile([128, NT_MAX], f32, tag="ps", name=f"p1_{s}_{m}_{j}")
                        for k in range(16):
                            nc.tensor.matmul(
                                p1[:, :nt], w1m[:, ds(k, 1), :],
                                bts[k][:, ds(n0, nt)],
                                start=(k == 0), stop=(k == 15),
                            )
                        nc.scalar.activation(
                            gt[:, ds(m, 1), ds(n0, nt)], p1[:, :nt],
                            mybir.ActivationFunctionType.Silu,
                        )
                        p2 = pp.tile([128, NT_MAX], f32, tag="ps", name=f"p2_{s}_{m}_{j}")
                        for k in range(16):
                            nc.tensor.matmul(
                                p2[:, :nt], w3m[:, ds(k, 1), :],
                                bts[k][:, ds(n0, nt)],
                                start=(k == 0), stop=(k == 15),
                            )
                        nc.vector.tensor_mul(
                            out=ht[:, ds(m, 1), ds(n0, nt)], in0=p2[:, :nt],
                            in1=gt[:, ds(m, 1), ds(n0, nt)],
                        )
                for h in range(16):
                    w2h = w2p.tile([128, 8, 128], bf, tag="w2", name=f"w2_{s}_{h}")
                    nc.sync.dma_start(w2h, w2t[s, h])
                    for j, (n0, nt) in enumerate(nts):
                        p3 = pp.tile([128, NT_MAX], f32, tag="ps", name=f"p3_{s}_{h}_{j}")
                        for k in range(8):
                            nc.tensor.matmul(
                                p3[:, :nt], w2h[:, ds(k, 1), :],
                                ht[:, ds(k, 1), ds(n0, nt)],
                                start=(k == 0), stop=(k == 7),
                            )
                        yo = yp.tile([128, NT_MAX], bf, tag="y", name=f"y_{s}_{h}_{j}")
                        nc.any.tensor_copy(out=yo[:, :nt], in_=p3[:, :nt])
                        nc.sync.dma_start(
                            yt[ds(h * 128, 128), ds(o + n0, nt)], yo[:, :nt],
                        )
    nc.compile()
    return nc


_GRAPH_CACHE = {}


def _prepare(x, w_gate, gate_bias, w1, w3, w2):
    """Host-side routing, packing, and per-core input staging."""
    x = np.asarray(x, np.float32)
    eidx, w = _route(x, np.asarray(w_gate, np.float32), np.asarray(gate_bias, np.float32))
    flat_e, tok, pos, valid, counts = _dispatch_indices(eidx)
    caps, assign = _pack(counts)
    S = len(caps)
    CT = int(sum(caps))
    offs = np.concatenate([[0], np.cumsum(caps)]).astype(np.int64)

    w1b = np.asarray(w1, np.float32).astype(BF16)
    w3b = np.asarray(w3, np.float32).astype(BF16)
    w2b = np.asarray(w2, np.float32).astype(BF16)
    xb = x.astype(BF16)

    # per-expert token lists in arrival order
    etoks = []
    for e in range(E):
        m = (flat_e == e) & valid
        etoks.append(tok[m])

    # route -> (core, column) lookup tables, built from segment spans
    core_of = np.zeros((E, C), np.int64)
    col_of = np.zeros((E, C), np.int64)

    in_maps = []
    for c in range(M_CORES):
        w1s = np.zeros((S, 8, 128, 16, 128), BF16)
        w3s = np.zeros((S, 8, 128, 16, 128), BF16)
        w2s = np.zeros((S, 16, 128, 8, 128), BF16)
        buf = np.zeros((H, CT), BF16)
        for s in range(S):
            e, st, sz = assign[c][s]
            # weights: w1/w3 [H, I] -> (m, p, k, i); w2 [I, H] -> (h, p, k, j)
            # w1/w3 [H, I]: (k p) h-split, (m i) I-split -> [m, p, k, i]
            w1s[s] = w1b[e].reshape(16, 128, 8, 128).transpose(2, 1, 0, 3)
            w3s[s] = w3b[e].reshape(16, 128, 8, 128).transpose(2, 1, 0, 3)
            # w2 [I, H]: (k p) I-split, (h j) H-split -> [h, p, k, j]
            w2s[s] = w2b[e].reshape(8, 128, 16, 128).transpose(2, 1, 0, 3)
            if sz > 0:
                toks = etoks[e][st : st + sz]
                o = int(offs[s])
                buf[:, o : o + sz] = xb[toks].T
                core_of[e, st : st + sz] = c
                col_of[e, st : st + sz] = o + np.arange(sz)
        in_maps.append({"w1t": w1s, "w3t": w3s, "w2t": w2s, "buft": buf})

    meta = dict(caps=caps, CT=CT, flat_e=flat_e, tok=tok, pos=pos, valid=valid,
                w=w, core_of=core_of, col_of=col_of)
    return in_maps, meta


def _combine(ys, meta):
    """ys: [M_CORES, H, CT] bf16 -> full [N, H] f32 output."""
    flat_e, pos, valid, w = meta["flat_e"], meta["pos"], meta["valid"], meta["w"]
    safe_pos = np.where(valid, pos, 0)
    core_idx = meta["core_of"][flat_e, safe_pos]
    col_idx = meta["col_of"][flat_e, safe_pos]
    contrib = ys[core_idx, :, col_idx].astype(np.float32)  # [N*K, H]
    wf = np.where(valid, w.reshape(-1), 0.0).astype(np.float32)
    out = (contrib * wf[:, None]).reshape(N, K, H).sum(axis=1)
    return out.astype(np.float32)


def kernel(x, w_gate, gate_bias, w1, w3, w2):
    in_maps, meta = _prepare(x, w_gate, gate_bias, w1, w3, w2)
    key = tuple(meta["caps"])
    if key not in _GRAPH_CACHE:
        _GRAPH_CACHE[key] = _build_graph(meta["caps"])
    nc = _GRAPH_CACHE[key]
    res = run_bass_kernel_spmd(nc, in_maps, core_ids=list(range(M_CORES)))
    ys = np.stack([np.asarray(res.results[c]["yt"]) for c in range(M_CORES)])
    return _combine(ys, meta)
